# revision 9
# baseline (speedup 1.0000x reference)
"""Trainium2 Bass kernel for nn_Decoder (2-layer LSTM decoder, T=512 steps).

Strategy: data-parallel over batch (256 -> 32 per core across 8 cores).
Per core, per timestep:
  - gates = matmul accumulation in PSUM, weights streamed as the moving
    operand (float32r, 1 cyc/row at N=512), transposed h state as the
    stationary operand (lhsT).
  - LSTM cell pointwise on ScalarE (sigmoid/tanh) + VectorE/GpSimd.
  - h'/c' transposed back via PE transpose for the next step's lhsT and
    for the output projection.
  - projection (faithful torch concat/reshape quirk: output row r<128
    mixes batches 2r,2r+1 of layer l=0; rows>=128 use layer 1) computed
    with strided APs over the transposed state, then sigmoid, then DMA.
Host side: precompute latent @ Wih0^T + biases (latent is constant over
time), permute gate order to [i,f,o,g] so sigmoid covers one contiguous
block, pre-transpose all weights, and reassemble the output.
"""

import numpy as np

H = 256
LAT = 128
D = 512
L = 2
B = 256
T = 512
NCORES = 8
BC = B // NCORES  # 32 batch rows per core

# gate permutation: torch order [i, f, g, o] -> ours [i, f, o, g]
# (sigmoid on first 3H, tanh on last H)
_PERM = np.concatenate([
    np.arange(0, H),          # i
    np.arange(H, 2 * H),      # f
    np.arange(3 * H, 4 * H),  # o
    np.arange(2 * H, 3 * H),  # g
])

_CACHE = {}


def _build(t_steps, mm_dtype_name="float32r"):
    import concourse.bacc as bacc
    import concourse.tile as tile
    from concourse import mybir

    fp32 = mybir.dt.float32
    mm_dt = getattr(mybir.dt, mm_dtype_name)
    Sigmoid = mybir.ActivationFunctionType.Sigmoid
    Tanh = mybir.ActivationFunctionType.Tanh

    def mmcast(ap):
        return ap

    nc = bacc.Bacc()

    # ---- DRAM parameters (per-core shard) ----
    xg0 = nc.declare_dram_parameter("XG0", [BC, 4 * H], mm_dt, isOutput=False)      # latent@Wih0^T + b0, permuted
    whh0t = nc.declare_dram_parameter("WHH0T", [H, 4 * H], mm_dt, isOutput=False)   # Whh0^T, cols permuted
    wih1t = nc.declare_dram_parameter("WIH1T", [H, 4 * H], mm_dt, isOutput=False)
    whh1t = nc.declare_dram_parameter("WHH1T", [H, 4 * H], mm_dt, isOutput=False)
    b1 = nc.declare_dram_parameter("B1", [1, 4 * H], mm_dt, isOutput=False)         # bih1+bhh1 permuted
    wmapt = nc.declare_dram_parameter("WMAPT", [4 * H, D], mm_dt, isOutput=False)   # Wmap^T
    bmap = nc.declare_dram_parameter("BMAP", [1, D], mm_dt, isOutput=False)
    ident = nc.declare_dram_parameter("I32", [BC, BC], mm_dt, isOutput=False)
    ones = nc.declare_dram_parameter("ONES", [1, BC], mm_dt, isOutput=False)
    identf = nc.declare_dram_parameter("I32F", [BC, BC], fp32, isOutput=False)
    out_p = nc.declare_dram_parameter("OUT", [t_steps, BC, D], fp32, isOutput=True)

    with tile.TileContext(nc) as tc:
        import contextlib
        with contextlib.ExitStack() as ctx:
            singles = ctx.enter_context(tc.tile_pool(name="singles", bufs=1))
            ht_pool = ctx.enter_context(tc.tile_pool(name="ht", bufs=3))
            ct_pool = ctx.enter_context(tc.tile_pool(name="ct", bufs=2))
            c_pool = ctx.enter_context(tc.tile_pool(name="c", bufs=4))
            g_pool = ctx.enter_context(tc.tile_pool(name="g", bufs=2))
            tmp_pool = ctx.enter_context(tc.tile_pool(name="tmp", bufs=6))
            h_pool = ctx.enter_context(tc.tile_pool(name="h", bufs=4))
            outs_pool = ctx.enter_context(tc.tile_pool(name="outs", bufs=3))
            ps_g = ctx.enter_context(tc.tile_pool(name="psg", bufs=2, space="PSUM"))
            ps_p = ctx.enter_context(tc.tile_pool(name="psp", bufs=2, space="PSUM"))
            ps_t = ctx.enter_context(tc.tile_pool(name="pst", bufs=2, space="PSUM"))

            # ---- load constants/weights into SBUF ----
            def load(pool, param, shape, nm):
                t_ = pool.tile(shape, mm_dt, name=nm, tag=nm)
                nc.sync.dma_start(out=t_, in_=param[:, :])
                return t_

            s_xg0 = load(singles, xg0, [BC, 4 * H], "sxg0")
            s_b1 = load(singles, b1, [1, 4 * H], "sb1")
            s_bmap = load(singles, bmap, [1, D], "sbmap")
            s_i32 = load(singles, ident, [BC, BC], "si32")
            s_ones = load(singles, ones, [1, BC], "sones")
            s_i32f = singles.tile([BC, BC], fp32, name="si32f", tag="si32f")
            nc.sync.dma_start(out=s_i32f, in_=identf[:, :])
            # weight K-tiles: (128, 1024) each
            s_whh0 = [singles.tile([128, 4 * H], mm_dt, name=f"whh0_{k}") for k in range(2)]
            s_wih1 = [singles.tile([128, 4 * H], mm_dt, name=f"wih1_{k}") for k in range(2)]
            s_whh1 = [singles.tile([128, 4 * H], mm_dt, name=f"whh1_{k}") for k in range(2)]
            for k in range(2):
                nc.sync.dma_start(out=s_whh0[k], in_=whh0t[128 * k:128 * k + 128, :])
                nc.sync.dma_start(out=s_wih1[k], in_=wih1t[128 * k:128 * k + 128, :])
                nc.sync.dma_start(out=s_whh1[k], in_=whh1t[128 * k:128 * k + 128, :])
            s_wmap = [singles.tile([128, D], mm_dt, name=f"wmap_{k}") for k in range(8)]
            for k in range(8):
                nc.sync.dma_start(out=s_wmap[k], in_=wmapt[128 * k:128 * k + 128, :])

            # ---- initial state ----
            ht_prev = ht_pool.tile([128, 128], mm_dt)  # [h0k0|h0k1|h1k0|h1k1] x 32 batch
            nc.vector.memset(ht_prev[:, :].bitcast(fp32), 0.0)
            c_prev = [c_pool.tile([BC, H], fp32, name=f"cinit{l}", tag=f"c{l}") for l in range(2)]
            for l in range(2):
                nc.vector.memset(c_prev[l], 0.0)

            three_h = 3 * H  # sigmoid block [i,f,o]

            for t in range(t_steps):
                ht_new = ht_pool.tile([128, 128], mm_dt, name=f"ht_{t}", tag="ht")
                ct_new = ct_pool.tile([128, 128], mm_dt, name=f"ct_{t}", tag="ct")
                htct_ps = ps_t.tile([128, 256], fp32, name=f"htct_{t}", tag="htct")  # [h0|h1|c0|c1] transposed
                c_new = [None, None]

                for l in range(2):
                    psg = ps_g.tile([BC, 4 * H], fp32, name=f"psg_{t}_{l}", tag="psg")
                    # ---- gate matmuls (weights moving, state stationary) ----
                    for ncn in range(2):
                        sl = slice(512 * ncn, 512 * ncn + 512)
                        if l == 0:
                            nc.tensor.matmul(psg[:, sl], mmcast(s_i32[:, :]),
                                             mmcast(s_xg0[:, sl]), start=True, stop=False)
                            for k in range(2):
                                nc.tensor.matmul(psg[:, sl],
                                                 mmcast(ht_prev[:, 64 * k:64 * k + 32]),
                                                 mmcast(s_whh0[k][:, sl]),
                                                 start=False, stop=(k == 1))
                        else:
                            nc.tensor.matmul(psg[:, sl], mmcast(s_ones[:, :]),
                                             mmcast(s_b1[:, sl]), start=True, stop=False)
                            for k in range(2):
                                nc.tensor.matmul(psg[:, sl],
                                                 mmcast(ht_new[:, 64 * k:64 * k + 32]),
                                                 mmcast(s_wih1[k][:, sl]),
                                                 start=False, stop=False)
                            for k in range(2):
                                nc.tensor.matmul(psg[:, sl],
                                                 mmcast(ht_prev[:, 64 * k + 32:64 * k + 64]),
                                                 mmcast(s_whh1[k][:, sl]),
                                                 start=False, stop=(k == 1))
                    # ---- LSTM cell pointwise ----
                    g_sb = g_pool.tile([BC, 4 * H], fp32, name=f"g_{t}_{l}", tag="g")
                    nc.scalar.activation(out=g_sb[:, 0:three_h], in_=psg[:, 0:three_h], func=Sigmoid)
                    nc.scalar.activation(out=g_sb[:, three_h:], in_=psg[:, three_h:], func=Tanh)
                    tig = tmp_pool.tile([BC, H], fp32, name=f"tig_{t}_{l}", tag="tig")
                    nc.vector.tensor_mul(tig, g_sb[:, 0:H], g_sb[:, three_h:])        # i*g
                    tfc = tmp_pool.tile([BC, H], fp32, name=f"tfc_{t}_{l}", tag="tfc")
                    nc.gpsimd.tensor_mul(tfc, g_sb[:, H:2 * H], c_prev[l])            # f*c
                    cn = c_pool.tile([BC, H], fp32, name=f"c_{t}_{l}", tag=f"c{l}")
                    nc.vector.tensor_add(cn, tig, tfc)                                # c'
                    c_new[l] = cn
                    tc_sb = tmp_pool.tile([BC, H], fp32, name=f"tcs_{t}_{l}", tag="tcs")
                    nc.scalar.activation(out=tc_sb, in_=cn, func=Tanh)
                    hn = h_pool.tile([BC, H], fp32, name=f"h_{t}_{l}", tag=f"h{l}")
                    nc.vector.tensor_mul(hn, g_sb[:, 2 * H:three_h], tc_sb)           # h' = o*tanh(c')
                    # ---- transpose h', c' (PE) into psum staging ----
                    for k in range(2):
                        nc.tensor.transpose(htct_ps[:, 64 * k + 32 * l:64 * k + 32 * l + 32],
                                            hn[:, 128 * k:128 * k + 128], s_i32f[:, :])
                        nc.tensor.transpose(htct_ps[:, 128 + 64 * k + 32 * l:128 + 64 * k + 32 * l + 32],
                                            cn[:, 128 * k:128 * k + 128], s_i32f[:, :])
                    # copy transposed h to SBUF promptly (layer1 matmuls need it)
                    for k in range(2):
                        nc.vector.tensor_copy(ht_new[:, 64 * k + 32 * l:64 * k + 32 * l + 32],
                                              htct_ps[:, 64 * k + 32 * l:64 * k + 32 * l + 32])
                nc.vector.tensor_copy(ct_new[:, :], htct_ps[:, 128:256])

                # ---- projection: out rows = [16 top pairs (layer0), 16 bottom (layer1)] ----
                psp = ps_p.tile([BC, D], fp32, name=f"psp_{t}", tag="psp")
                nc.tensor.matmul(psp[:, :], mmcast(s_ones[:, :]), mmcast(s_bmap[:, :]),
                                 start=True, stop=False)
                def proj_lhs(tile_, k, parity):
                    seg = tile_[:, 64 * k:64 * k + 64]
                    return seg.rearrange("p (m two) -> p m two", two=2)[:, :, parity]
                # K-block order matches Wmap^T rows:
                # 0-255 h-even, 256-511 c-even, 512-767 h-odd, 768-1023 c-odd
                order = [(ht_new, 0, 0), (ht_new, 0, 1), (ct_new, 0, 0), (ct_new, 0, 1),
                         (ht_new, 1, 0), (ht_new, 1, 1), (ct_new, 1, 0), (ct_new, 1, 1)]
                for idx, (tile_, parity, k) in enumerate(order):
                    nc.tensor.matmul(psp[:, :],
                                     mmcast(proj_lhs(tile_, k, parity)),
                                     mmcast(s_wmap[idx][:, :]),
                                     start=False, stop=(idx == 7))
                out_sb = outs_pool.tile([BC, D], fp32, name=f"outsb_{t}", tag="outsb")
                nc.scalar.activation(out=out_sb, in_=psp, func=Sigmoid)
                nc.sync.dma_start(out=out_p[t, :, :], in_=out_sb)

                ht_prev = ht_new
                c_prev = c_new

    nc.finalize()
    return nc


def _prep_inputs(latent, Wih0, Whh0, bih0, bhh0, Wih1, Whh1, bih1, bhh1, Wmap, bmap):
    """Host-side preprocessing -> per-core in_maps."""
    latent = np.asarray(latent, np.float32)
    xg0_full = latent @ np.asarray(Wih0, np.float32).T + np.asarray(bih0, np.float32) + np.asarray(bhh0, np.float32)
    xg0_full = xg0_full[:, _PERM]                      # (B, 4H) permuted
    whh0t = np.asarray(Whh0, np.float32).T[:, _PERM].copy()   # (H, 4H)
    wih1t = np.asarray(Wih1, np.float32).T[:, _PERM].copy()
    whh1t = np.asarray(Whh1, np.float32).T[:, _PERM].copy()
    b1 = (np.asarray(bih1, np.float32) + np.asarray(bhh1, np.float32))[_PERM].reshape(1, 4 * H).copy()
    wmapt = np.asarray(Wmap, np.float32).T.copy()      # (4H=2*2H? no: (2LH=1024), D)
    bmap_r = np.asarray(bmap, np.float32).reshape(1, D).copy()
    i32 = np.eye(BC, dtype=np.float32)
    ones = np.ones((1, BC), np.float32)

    in_maps = []
    for c in range(NCORES):
        sl = slice(BC * c, BC * c + BC)
        in_maps.append({
            "XG0": np.ascontiguousarray(xg0_full[sl]),
            "WHH0T": whh0t, "WIH1T": wih1t, "WHH1T": whh1t, "B1": b1,
            "WMAPT": wmapt, "BMAP": bmap_r, "I32": i32, "ONES": ones, "I32F": i32,
        })
    return in_maps


def run(t_steps, inputs, mm_dtype_name="float32r", trace=False):
    import time as _time
    from concourse.bass_utils import run_bass_kernel_spmd

    key = (t_steps, mm_dtype_name)
    if key not in _CACHE:
        _t0 = _time.time()
        _CACHE[key] = _build(t_steps, mm_dtype_name)
        print(f"[kernel] build+finalize: {_time.time()-_t0:.1f}s", flush=True)
    nc = _CACHE[key]
    in_maps = _prep_inputs(
        inputs["latent"], inputs["Wih0"], inputs["Whh0"], inputs["bih0"], inputs["bhh0"],
        inputs["Wih1"], inputs["Whh1"], inputs["bih1"], inputs["bhh1"],
        inputs["Wmap"], inputs["bmap"])
    _t1 = _time.time()
    res = run_bass_kernel_spmd(nc, in_maps, list(range(NCORES)), trace=trace)
    print(f"[kernel] compile+exec: {_time.time()-_t1:.1f}s", flush=True)
    # reassemble: core c local pair p -> global row 16c+p (top) / 128+16c+p (bottom)
    out = np.empty((t_steps, B, D), np.float32)
    for c in range(NCORES):
        o = res.results[c]["OUT"]  # (T, 32, 512)
        out[:, 16 * c:16 * c + 16, :] = o[:, 0:16, :]
        out[:, 128 + 16 * c:128 + 16 * c + 16, :] = o[:, 16:32, :]
    return out, res


def kernel(**inputs):
    t_steps = int(inputs.get("seq_length", T))
    out, _ = run(t_steps, inputs)
    return out


if __name__ == "__main__":
    import argparse, time
    ap = argparse.ArgumentParser()
    ap.add_argument("--t", type=int, default=8)
    ap.add_argument("--dt", type=str, default="float32r")
    args = ap.parse_args()

    rng = np.random.default_rng(0)
    s = 1.0 / np.sqrt(H)
    sm = 1.0 / np.sqrt(2 * L * H)
    inputs = {
        "latent": rng.standard_normal((B, LAT), np.float32),
        "Wih0": rng.uniform(-s, s, (4 * H, LAT)).astype(np.float32),
        "Whh0": rng.uniform(-s, s, (4 * H, H)).astype(np.float32),
        "bih0": rng.uniform(-s, s, (4 * H,)).astype(np.float32),
        "bhh0": rng.uniform(-s, s, (4 * H,)).astype(np.float32),
        "Wih1": rng.uniform(-s, s, (4 * H, H)).astype(np.float32),
        "Whh1": rng.uniform(-s, s, (4 * H, H)).astype(np.float32),
        "bih1": rng.uniform(-s, s, (4 * H,)).astype(np.float32),
        "bhh1": rng.uniform(-s, s, (4 * H,)).astype(np.float32),
        "Wmap": rng.uniform(-sm, sm, (D, 2 * L * H)).astype(np.float32),
        "bmap": rng.uniform(-sm, sm, (D,)).astype(np.float32),
        "seq_length": args.t,
    }

    def np_ref(inp, t_steps):
        def sig(x):
            return 1.0 / (1.0 + np.exp(-x))
        h = np.zeros((L, B, H), np.float32)
        c = np.zeros((L, B, H), np.float32)
        xg = inp["latent"] @ inp["Wih0"].T + inp["bih0"] + inp["bhh0"]
        outs = []
        for _ in range(t_steps):
            for l in range(L):
                x = (xg if l == 0 else h[0] @ inp["Wih1"].T + inp["bih1"] + inp["bhh1"])
                gates = x if l == 0 else x
                if l == 0:
                    gates = xg + h[0] @ inp["Whh0"].T
                else:
                    gates = h[0] @ inp["Wih1"].T + h[1] @ inp["Whh1"].T + inp["bih1"] + inp["bhh1"]
                i, f, g, o = np.split(gates, 4, axis=-1)
                i, f, o = sig(i), sig(f), sig(o)
                g = np.tanh(g)
                c[l] = f * c[l] + i * g
                h[l] = o * np.tanh(c[l])
            fc = np.concatenate([h, c], axis=-1).reshape(B, -1)
            outs.append(sig(fc @ inp["Wmap"].T + inp["bmap"]))
        return np.stack(outs)

    t0 = time.time()
    out, res = run(args.t, inputs, args.dt)
    print("total wall:", round(time.time() - t0, 1), "s")
    ref = np_ref(inputs, args.t)
    err = np.abs(out - ref)
    denom = max(1e-6, np.abs(ref).max())
    print("abs max err:", err.max(), " rel:", err.max() / denom)
